# revision 4
# baseline (speedup 1.0000x reference)
"""Trainium2 Bass kernel for nn_DenseAttentionLayer (gnn_message_passing).

Math (reference):
    in_fts = context @ W_common.T            # (N, HID)
    left   = in_fts @ w_left + b_left        # (N,)
    right  = in_fts @ w_right + b_right      # (N,)
    logits = leaky_relu(left[:,None] + right[None,:], 0.2)
    logits = where(adj <= 0, -inf, logits)
    coefs  = softmax(logits, axis=-1)
    out    = relu(coefs @ relation)          # (N, REL_DIM)

Key identity: softmax over j is invariant to any per-row scale, and the
logits are rank-1 (x_ij = L_i + R_j), so scaling row i by
exp(-0.2 L_i - C):

    exp(leaky(x_ij)) * s_i = max(exp(x), exp(0.2 x)) * s_i
                           = max( A_i * b_j , d_j )
    A_i = exp(0.8 L_i - C),  b_j = exp(R_j),  d_j = exp(0.2 R_j - C)

The N x N tile work therefore needs NO exp at all -- one tensor_scalar
(mult+max against per-partition scalars b_j, d_j; DVE 4x fp16 mode) and
one tensor_tensor multiply with the 0/1 adjacency (DVE 2x fp16 mode).
exp runs only on N-vectors.  Masked entries become exactly 0, and the
softmax denominator comes free as column 256 of the P@V matmul
(relation augmented with a ones column).

Layout: TRANSPOSED -- j on partitions, i on the free dim.  The host
pre-transposes each core's adjacency row-shard (adj[rows].T in fp16) so
zm tiles are produced directly in lhsT form for the accumulating
matmul: NO PE transposes.  acc[i, d] += sum_j zm[j, i] * rel_aug[j, d]
accumulates across all 64 j-tiles into 8 PSUM banks (one per i-block,
512-f32 stride => bank aligned).

The left/right dot products (ctx . v) run on the PE against a
host-transposed fp16 ctx.T (lhsT = [128f x 128j] tiles, rhs = v
reshaped [128, nf]), accumulating the 4 f-tiles into a small region of
PSUM bank 0 that overlaps the ib=0 chain span: the WAR/WAW edges from
that overlap order all dots + their exp reads before the chains' first
(span-zeroing) matmul, which keeps the PE stream as one clean run of
dot groups followed by one clean run of chain matmuls (interleaving
the two kinds of accumulation groups measured ~15% slower on HW).

Sharding (8 cores): row-shard the N x N logits; context/relation/
params replicated.  Per core the host rolls the j axis so the core's
own rows land first (softmax sums over j, so any per-core j
permutation is valid when adjT rows / ctxT cols / rel rows get the
same permutation) -- the left dots then read the same resident ctx.T
as the right dots, and no separate own-context input is needed.
Host-side prep is dtype casts / transposes / weight folds only -- no
activation math on host.
"""

import os
import sys

for _p in ("/opt/trn_rl_repo",):
    if _p not in sys.path and os.path.isdir(_p):
        sys.path.insert(0, _p)

from contextlib import ExitStack

import numpy as np

# ---------------------------------------------------------------- constants
N = 8192  # num relations
IN = 512  # context feature dim
D = 256  # relation dim (output dim)
NCORES = 8
P = 128
CSHIFT = 5.0  # global exponent shift (cancels in softmax)

_CACHE = {}


# ------------------------------------------------------------------ builder
def build_program(cfg):
    import concourse.bass as bass
    import concourse.tile as tile
    from concourse import bacc, mybir

    f32 = mybir.dt.float32
    f16 = mybir.dt.float16

    n = cfg["n"]  # full N (j extent)
    r = cfg["r"]  # rows per core (i extent)
    reps = cfg.get("reps", 1)

    nt = n // P  # j-tiles (64)
    ni = r // P  # i-blocks (8)
    nf = IN // P  # f-tiles (4)
    JC = cfg.get("jc", 4)  # j-tiles per adj DMA chunk
    nch = nt // JC  # adj chunks (16)
    DOTB = cfg.get("dotb", 8)  # j-tiles per right-dot batch

    nc = bacc.Bacc("TRN2", target_bir_lowering=False, debug=False)

    # per-core inputs
    adjT = nc.dram_tensor("adjT", [n, r], f16, kind="ExternalInput")
    ctxT = nc.dram_tensor("ctxT", [IN, n], f16, kind="ExternalInput")
    rel_in = nc.dram_tensor("rel_in", [n, D], f16, kind="ExternalInput")
    vl_in = nc.dram_tensor("vl_in", [IN], f16, kind="ExternalInput")
    vr_in = nc.dram_tensor("vr_in", [IN], f16, kind="ExternalInput")
    # pars = [b_r, 0.2*b_r - C, 0.8*b_l - C]
    pars = nc.dram_tensor("pars", [3], f32, kind="ExternalInput")
    out = nc.dram_tensor("out", [r, D], f16, kind="ExternalOutput")
    a_scr = nc.dram_tensor("a_scratch", [r], f16)

    alu = mybir.AluOpType
    act = mybir.ActivationFunctionType

    with tile.TileContext(nc) as tc, ExitStack() as ctx:
        singles = ctx.enter_context(tc.tile_pool(name="singles", bufs=1))
        rel_pool = ctx.enter_context(tc.tile_pool(name="relp", bufs=2))
        vec_pool = ctx.enter_context(tc.tile_pool(name="vecp", bufs=2))
        adj_pool = ctx.enter_context(
            tc.tile_pool(name="adjp", bufs=cfg.get("adj_bufs", 4))
        )
        zp_pool = ctx.enter_context(tc.tile_pool(name="zpp", bufs=2))
        zm_pool = ctx.enter_context(
            tc.tile_pool(name="zmp", bufs=cfg.get("zm_bufs", 8))
        )
        sm_pool = ctx.enter_context(tc.tile_pool(name="smp", bufs=2))
        acc_psum = ctx.enter_context(tc.tile_pool(name="accps", bufs=1, space="PSUM"))

        def _emit_body():
            # ---------------- phase 0: params ----------------
            # v vectors in f-tile form: v2[p, q] = v[q*128 + p]
            vlb2 = singles.tile([P, nf], f16)
            nc.sync.dma_start(
                out=vlb2, in_=bass.AP(tensor=vl_in, offset=0, ap=[[1, P], [P, nf]])
            )
            vrb2 = singles.tile([P, nf], f16)
            nc.sync.dma_start(
                out=vrb2, in_=bass.AP(tensor=vr_in, offset=0, ap=[[1, P], [P, nf]])
            )
            bias_b = singles.tile([P, 1], f32)
            nc.sync.dma_start(
                out=bias_b, in_=bass.AP(tensor=pars, offset=0, ap=[[0, P], [1, 1]])
            )
            bias_d = singles.tile([P, 1], f32)
            nc.sync.dma_start(
                out=bias_d, in_=bass.AP(tensor=pars, offset=1, ap=[[0, P], [1, 1]])
            )
            bias_a = singles.tile([P, 1], f32)
            nc.sync.dma_start(
                out=bias_a, in_=bass.AP(tensor=pars, offset=2, ap=[[0, P], [1, 1]])
            )

            # relation, augmented with a ones column (denominator trick).
            # Double-buffered: the reload for the next For_i iteration starts
            # while this iteration's matmuls still read the other slot.
            rel_aug = rel_pool.tile([P, nt, D + 1], f16, tag="rel")
            nc.vector.memset(rel_aug[:, :, D : D + 1], 1.0)
            nc.sync.dma_start(
                out=rel_aug[:, :, 0:D],
                in_=rel_in.ap().rearrange("(t p) d -> p t d", p=P),
            )

            # ctx.T resident in SBUF: [p_f, q, j]
            ctxT_sb = singles.tile([P, nf, n], f16)
            for jc in range(4):
                w = n // 4
                for q in range(nf):
                    nc.sync.dma_start(
                        out=ctxT_sb[:, q, jc * w : (jc + 1) * w],
                        in_=ctxT[q * P : (q + 1) * P, jc * w : (jc + 1) * w],
                    )

            b_cols = vec_pool.tile([P, nt], f32, tag="b_cols")
            d_cols = vec_pool.tile([P, nt], f32, tag="d_cols")
            a_cols = vec_pool.tile([P, ni], f16, tag="a_cols")
            a_b = vec_pool.tile([P, r], f16, tag="a_b")

            # acc[:, ib, 0:257] accumulate the P@V result over all 64 j-tiles
            # (one PSUM bank per i-block).  All dot products run in a
            # pre-phase in ONE region of bank 0 overlapping the ib=0 chain:
            # the overlap orders every dot write + exp read before the
            # chains' first (zeroing) matmul.
            acc = acc_psum.tile([P, ni, 512], f32)
            dreg = acc[:, 0, 0:DOTB]

            # ---- left dots on PE: L[t] for own rows (tiles 0..ni after the
            # host j-roll), i-order on partitions
            for t in range(ni):
                for q in range(nf):
                    nc.tensor.matmul(
                        acc[:, 0, t : t + 1],
                        lhsT=ctxT_sb[:, q, t * P : (t + 1) * P],
                        rhs=vlb2[:, q : q + 1],
                        start=(q == 0),
                        stop=(q == nf - 1),
                        skip_group_check=True,
                    )
            # A = exp(0.8 L + (0.8 b_l - C)), bounced via DRAM into broadcast
            nc.scalar.activation(
                a_cols, acc[:, 0, 0:ni], act.Exp, bias=bias_a[:, 0:1], scale=0.8
            )
            nc.sync.dma_start(
                out=bass.AP(tensor=a_scr, offset=0, ap=[[1, P], [P, ni]]),
                in_=a_cols[:, 0:ni],
            )
            nc.sync.dma_start(
                out=a_b, in_=bass.AP(tensor=a_scr, offset=0, ap=[[0, P], [1, r]])
            )

            # ---- right dots on PE, batches of DOTB j-tiles, same region
            for k in range(nt // DOTB):
                for t in range(DOTB):
                    jt = k * DOTB + t
                    for q in range(nf):
                        nc.tensor.matmul(
                            dreg[:, t : t + 1],
                            lhsT=ctxT_sb[:, q, jt * P : (jt + 1) * P],
                            rhs=vrb2[:, q : q + 1],
                            start=(q == 0),
                            stop=(q == nf - 1),
                            skip_group_check=True,
                        )
                # b = exp(R + b_r), d = exp(0.2 R + 0.2 b_r - C)
                sl = slice(k * DOTB, (k + 1) * DOTB)
                nc.scalar.activation(
                    b_cols[:, sl], dreg, act.Exp, bias=bias_b[:, 0:1], scale=1.0
                )
                nc.scalar.activation(
                    d_cols[:, sl], dreg, act.Exp, bias=bias_d[:, 0:1], scale=0.2
                )

            # ------------------------- main loop ----------------------------
            for k in range(nch):
                # adjacency: JC j-tiles per DMA (2KB lines)
                adj4 = adj_pool.tile([P, JC, r], f16, tag="adj")
                nc.sync.dma_start(
                    out=adj4,
                    in_=adjT.ap().rearrange("(c t p) i -> c p t i", c=nch, p=P)[k],
                )
                for t in range(JC):
                    jt = k * JC + t
                    # zp = max(A_i * b_j, d_j)  (DVE 4x fp16)
                    zp = zp_pool.tile([P, r], f16, tag="zp")
                    nc.vector.tensor_scalar(
                        zp, a_b, b_cols[:, jt : jt + 1], d_cols[:, jt : jt + 1],
                        alu.mult, alu.max,
                    )
                    # zm = zp * adj  (exact 0 for masked; DVE 2x fp16)
                    zm = zm_pool.tile([P, r], f16, tag="zm")
                    nc.vector.tensor_mul(zm, zp, adj4[:, t, :])
                    for ib in range(ni):
                        nc.tensor.matmul(
                            acc[:, ib, 0 : D + 1],
                            lhsT=zm[:, ib * P : (ib + 1) * P],
                            rhs=rel_aug[:, jt, :],
                            start=(jt == 0),
                            stop=(jt == nt - 1),
                            skip_group_check=True,
                        )

            # ---------------------- epilogue -------------------------------
            ob_all = singles.tile([P, ni, D], f16)
            for ib in range(ni):
                recip = sm_pool.tile([P, 1], f32, tag="recip")
                nc.vector.reciprocal(recip, acc[:, ib, D : D + 1])
                nc.scalar.activation(
                    ob_all[:, ib, :], acc[:, ib, 0:D], act.Relu,
                    bias=0.0, scale=recip[:, 0:1],
                )
            nc.sync.dma_start(
                out=out.ap().rearrange("(b p) d -> p b d", p=P), in_=ob_all
            )

        if reps > 1:
            with tc.For_i(0, reps, 1):
                _emit_body()
        else:
            _emit_body()

    nc.compile()
    return nc


_BASE_CFG = dict(n=N, r=N // NCORES)


def _get_program(cfg_key):
    if cfg_key not in _CACHE:
        _CACHE[cfg_key] = build_program(dict(_BASE_CFG))
    return _CACHE[cfg_key]


def prepare_in_maps(relation, context, adj_tensor, W_common, w_left, b_left,
                    w_right, b_right):
    relation = np.asarray(relation, dtype=np.float32)
    context = np.asarray(context, dtype=np.float32)
    adj_tensor = np.asarray(adj_tensor, dtype=np.float32)
    W_common = np.asarray(W_common, dtype=np.float32)
    w_left = np.asarray(w_left, dtype=np.float32)
    w_right = np.asarray(w_right, dtype=np.float32)
    b_l = float(np.asarray(b_left))
    b_r = float(np.asarray(b_right))

    # host-side parameter folding (weights only, no activations)
    v_left = (W_common.T @ w_left).astype(np.float32)
    v_right = (W_common.T @ w_right).astype(np.float32)
    pars = np.array(
        [b_r, 0.2 * b_r - CSHIFT, 0.8 * b_l - CSHIFT], dtype=np.float32
    )

    ctxT16 = context.T.astype(np.float16)  # [IN, N]
    rel16 = relation.astype(np.float16)
    adj16 = adj_tensor.astype(np.float16)

    rows = N // NCORES
    in_maps = []
    for c in range(NCORES):
        sl = slice(c * rows, (c + 1) * rows)
        # roll the j axis so this core's own rows land at j' in [0, rows):
        # softmax sums over j, so any per-core j permutation is valid as long
        # as adjT rows, ctxT columns and rel rows are permuted identically.
        perm = np.roll(np.arange(N), -c * rows)
        m = {
            "adjT": np.ascontiguousarray(adj16[sl].T[perm]),
            "ctxT": np.ascontiguousarray(ctxT16[:, perm]),
            "rel_in": np.ascontiguousarray(rel16[perm]),
            "vl_in": v_left.astype(np.float16),
            "vr_in": v_right.astype(np.float16),
            "pars": pars,
        }
        in_maps.append(m)
    return in_maps


# ------------------------------------------------------------------- entry
def kernel(relation, context, adj_tensor, W_common, w_left, b_left, w_right,
           b_right):
    from concourse.bass_utils import run_bass_kernel_spmd

    in_maps = prepare_in_maps(relation, context, adj_tensor, W_common,
                              w_left, b_left, w_right, b_right)
    nc = _get_program("main")
    last_err = None
    for _attempt in range(3):
        try:
            res = run_bass_kernel_spmd(nc, in_maps, list(range(NCORES)))
            outs = [res.results[c]["out"] for c in range(NCORES)]
            return np.concatenate(outs, axis=0).astype(np.float32)
        except Exception as e:  # transient device-unrecoverable seen on axon
            last_err = e
            import time as _time

            try:
                import jax

                jax.clear_caches()
            except Exception:
                pass
            _time.sleep(3.0)
    raise last_err
